# revision 1
# baseline (speedup 1.0000x reference)
"""Trainium2 Bass kernel for the LocalConnectivity diamond-ring stencil.

out[b, x, y] = sum_{1<=|dx|+|dy|<=5} w[|dx|+|dy|-1] * in[b, (x+dx)%512, (y+dy)%512]

Measured HW exec: ~99.6 us on 8 trn2 cores (staged baseline: 218.9 us).

Strategy
--------
Data-parallel over batch: 64 samples -> 8 cores x 8 samples. Per sample the
512x512 grid is processed in 5 uniform row-tiles of 103 output rows, all
identical in shape (the 5th tile computes 3 circularly-wrapped rows the
host drops), sharing 9 banded fp16 weight matrices.

The stencil splits across engines:
 - TensorE: 9 PSUM-accumulating matmuls per tile, one per horizontal shift
   dy in [-4, 4]: psum[p,f] += sum_c WB_dy[c,p] * X[c, f+dy_idx], vertical
   taps in a banded Toeplitz stationary (contraction over 113 input rows),
   horizontal shifts as free-dim AP offsets. The band is ALIGNED (out row
   103t+p-5 at psum partition p = the xt partition holding that row), which
   is what lets the dy=+-5 columns leave the PE. dy-outer ordering reuses
   each stationary across tiles; warm rate 216 ns/matmul, stream ~78 us.
 - VectorE: the two single-tap dy=+-5 columns ride the PSUM eviction:
   pass A (during the matmul stream, xt-only dep): tmp = x[r,y-5]+x[r,y+5]
   read at the SAME partition; pass B: otb = psum + w5*tmp via
   scalar_tensor_tensor, with w5 read per-partition from an extra column
   of the weight tile (runtime value, can't be an immediate).
 - 16 dummy matmuls during the load warm the HAM clock-gate (cold = half
   clock). First and last samples run t-outer: the first chases arriving
   input planes, the last staggers psum completion so eviction + output
   drain inside the stream.

Hardware facts this layout is built on (all microbenchmarked here):
 - gpsimd software-DGE DMAs whose SBUF side spans all 128 partitions run
   at 300-470 GB/s; ANY partial partition range takes a degraded ~43 GB/s
   path on 2 SDMA engines. Every bulk transfer is a full-128-partition DMA:
    * INPUT host-padded to [8, 540, 512]/core (5 wrap rows front, 23 back)
      so each tile-plane is one clean [128, gsz, 512] f32->fp16 casting
      DMA, no wrap/halo fixups; sample-groups (2,3,3) stagger wire time.
    * OUTPUT to padded y [8, 5*128, 512]: one [128, 5, 512] DMA per sample
      (rows outside partitions 5..107 are garbage the host slices off);
      last two samples split 3 ways so the tail drains early.
 - Each gpsimd dma_start costs ~1 us fixed on the Q7 and the 8 DMA
   completion semaphores recycle round-robin (a 9th in-flight DMA blocks
   the queue) -> only ~25 DMA instructions total.
 - sync/scalar hardware-DGE rings are ~11-18 GB/s serial paths; useless
   for bulk. Circular column halos are on-chip ScalarE copies.
"""

import numpy as np

import concourse.bass as bass
import concourse.bacc as bacc
import concourse.mybir as mybir
from concourse import tile
from concourse.bass_utils import run_bass_kernel_spmd

B, H, W = 64, 512, 512
NCORES = 8
BPC = B // NCORES  # samples per core
MAXD = 5
HALO = MAXD
DYS = 2 * MAXD + 1  # 11 horizontal shifts
TR = 103  # output rows per tile
NT = 5
CTR = TR + 2 * HALO  # 113 contraction rows
XW = W + 2 * HALO  # 522
HPAD = HALO + H + 23  # 540 padded input rows per sample
HOUT = NT * 128  # 640 padded output rows per sample

IN_GROUPS = [(0, 2), (2, 3), (5, 3)]  # (b0, size); staggered to hide wire time

F16 = mybir.dt.float16


def _build_band_weights(dw: np.ndarray) -> np.ndarray:
    """[128, 11*128]: WB[c, j*128 + p] = K(c-p, j-5) (band aligned so psum
    partition p holds output row 103t + p - 5, matching xt partitions)."""
    wb = np.zeros((128, DYS, 128), dtype=np.float32)
    p = np.arange(128)
    for j in range(DYS):
        dy = j - MAXD
        for dx in range(-MAXD, MAXD + 1):
            d = abs(dx) + abs(dy)
            if 1 <= d <= MAXD:
                c = p + dx
                valid = (c >= 0) & (c < 128)
                wb[c[valid], j, p[valid]] = dw[d - 1]
    flat = wb.reshape(128, DYS * 128)
    w5col = np.full((128, 1), dw[MAXD - 1], dtype=np.float32)
    return np.ascontiguousarray(
        np.concatenate([flat, w5col], axis=1).astype(np.float16)
    )


_CACHED_NC = None


def _custom_ap(base_ap, dims, extra_offset_elems=0):
    """Build a strided AP: dims = [(stride_elems, size), ...]."""
    s = base_ap.copy()
    s.ap.clear()
    s.ap.extend(dims)
    s.offset = s.offset + extra_offset_elems
    return s


def _build_program():
    f32 = mybir.dt.float32

    nc = bacc.Bacc(None, target_bir_lowering=False)
    x = nc.dram_tensor("x", [BPC, HPAD, W], f32, kind="ExternalInput")
    wb = nc.dram_tensor("wb", [128, DYS * 128 + 1], F16, kind="ExternalInput")
    y = nc.dram_tensor("y", [BPC, HOUT, W], f32, kind="ExternalOutput")

    with tile.TileContext(nc) as tc:
        with (
            tc.tile_pool(name="wpool", bufs=1) as wpool,
            tc.tile_pool(name="xpool_a", bufs=1) as xpool_a,
            tc.tile_pool(name="xpool_b", bufs=2) as xpool_b,
            tc.tile_pool(name="opool", bufs=3) as opool,
            tc.tile_pool(name="tpool", bufs=8) as tpool,
            tc.tile_pool(name="pspool", bufs=8, space=bass.MemorySpace.PSUM) as pspool,
        ):
            wtile = wpool.tile([128, DYS * 128 + 1], F16, tag="wt")
            nc.gpsimd.dma_start(wtile[:], wb[:])

            # PE warm-up: 16 dummy matmuls during the input-load window
            # trip the HAM clock-gate to 8/8 before the real stream starts.
            # Empirically optimal: 10 measured +2us, 2 measured +3.3us (the
            # cold ramp on real matmuls costs more than idling warm)
            dummy = wpool.tile([128, 640], F16, tag="dummy")
            nc.vector.memset(dummy[:], 0.0)
            wpt = pspool.tile([128, W], mybir.dt.float32, tag="pt")
            for _ in range(16):
                nc.tensor.matmul(wpt[0:TR, :], dummy[0:CTR, 0:TR],
                                 dummy[0:CTR, 64:576], start=True, stop=True)

            sample_xt = {}
            for gi, (pool, (b0, gsz)) in enumerate(
                zip((xpool_a, xpool_b, xpool_b), IN_GROUPS)
            ):
                # xt[p, b, t, 5+y] = xpad[b0+b, 103t + p, y]
                #                  = x_orig[b0+b, (103t - 5 + p) % 512, y]
                xt = pool.tile([128, gsz, NT, XW], F16,
                               tag="xta" if gi == 0 else "xtbc")
                for t in range(NT):
                    src = _custom_ap(
                        x[b0], [(W, 128), (HPAD * W, gsz), (1, W)],
                        extra_offset_elems=TR * t * W,
                    )
                    nc.gpsimd.dma_start(xt[:, :, t, HALO : HALO + W], src)
                    # circular column halos for this plane
                    nc.scalar.copy(
                        xt[0:CTR, :, t, 0:HALO], xt[0:CTR, :, t, W : W + HALO]
                    )
                    nc.scalar.copy(
                        xt[0:CTR, :, t, HALO + W :],
                        xt[0:CTR, :, t, HALO : 2 * HALO],
                    )
                for bi in range(gsz):
                    sample_xt[b0 + bi] = (xt, bi)

            for b in range(BPC):
                xt, bq = sample_xt[b]
                # ---- 45 matmuls: dy-outer, stationary reused over tiles ----
                pts = []
                tmps = []
                for t in range(NT):
                    pt = pspool.tile([128, W], f32, tag="pt")
                    pts.append(pt)
                    # pass A of the fused eviction: tmp = xl + xr (xt-only
                    # dependency; runs during the matmul stream)
                    tmp = tpool.tile([128, W], f32, tag="tmp")
                    nc.vector.scalar_tensor_tensor(
                        tmp[0:108, :],
                        xt[0:108, bq, t, 0:W],
                        0.0,
                        xt[0:108, bq, t, 10 : 10 + W],
                        mybir.AluOpType.bypass,
                        mybir.AluOpType.add,
                    )
                    tmps.append(tmp)
                if b == 0 or b == BPC - 1:
                    # t-outer: first sample chases arriving planes; last
                    # sample staggers psum completion so eviction + output
                    # drain during the stream instead of after it
                    loop = [(j, t) for t in range(NT) for j in range(1, DYS - 1)]
                else:
                    loop = [(j, t) for j in range(1, DYS - 1) for t in range(NT)]
                for j, t in loop:
                    nc.tensor.matmul(
                        pts[t][0:108, :],
                        wtile[0:CTR, j * 128 : j * 128 + 108],
                        xt[0:CTR, bq, t, j : j + W],
                        start=(j == 1),
                        stop=(j == DYS - 2),
                    )

                # ---- fused eviction: otb = psum + w5*(x[r,y-5]+x[r,y+5]);
                # the dy=+-5 taps read xt at the SAME partition (shifted
                # band). pass A (the tap sum) depends only on xt, so it was
                # issued before the matmuls; pass B folds it into psum. ----
                otb = opool.tile([128, NT, W], f32, tag="otb")
                w5ap = wtile[0:108, DYS * 128 : DYS * 128 + 1]
                for t in range(NT):
                    nc.vector.scalar_tensor_tensor(
                        otb[0:108, t, :],
                        tmps[t][0:108, :],
                        w5ap,
                        pts[t][0:108, :],
                        mybir.AluOpType.mult,
                        mybir.AluOpType.add,
                    )

                # ---- full-128-partition output DMAs ----
                if b >= BPC - 2:
                    for lo, hi in ((0, 2), (2, 4), (4, 5)):
                        dstp = _custom_ap(
                            y[b], [(W, 128), (128 * W, hi - lo), (1, W)],
                            extra_offset_elems=lo * 128 * W,
                        )
                        nc.gpsimd.dma_start(dstp, otb[:, lo:hi, :])
                else:
                    dst = _custom_ap(y[b], [(W, 128), (128 * W, NT), (1, W)])
                    nc.gpsimd.dma_start(dst, otb[:, :, :])
    nc.compile()
    return nc


def _get_program():
    global _CACHED_NC
    if _CACHED_NC is None:
        _CACHED_NC = _build_program()
    return _CACHED_NC


def _run(grid_spikes, distance_weights, trace=False):
    grid_spikes = np.ascontiguousarray(np.asarray(grid_spikes, dtype=np.float32))
    distance_weights = np.asarray(distance_weights, dtype=np.float32)
    assert grid_spikes.shape == (B, H, W), grid_spikes.shape
    wb_np = _build_band_weights(distance_weights)

    # pad each sample: 5 wrap rows in front (507..511), 23 behind (0..22)
    xpad = np.concatenate(
        [grid_spikes[:, H - HALO :, :], grid_spikes, grid_spikes[:, :23, :]],
        axis=1,
    )  # [B, 540, W]
    assert xpad.shape[1] == HPAD

    nc = _get_program()
    in_maps = [
        {
            "x": np.ascontiguousarray(xpad[i * BPC : (i + 1) * BPC]),
            "wb": wb_np,
        }
        for i in range(NCORES)
    ]
    res = run_bass_kernel_spmd(nc, in_maps, list(range(NCORES)), trace=trace)
    ypad = np.concatenate(
        [res.results[i]["y"] for i in range(NCORES)], axis=0
    )  # [B, 640, W]
    # unpack: row 103t + p lives at padded row 128t + p (p < 103); the last
    # tile's rows 512..514 are circular duplicates the slice drops.
    out = (
        ypad.reshape(B, NT, 128, W)[:, :, HALO : HALO + TR, :]
        .reshape(B, NT * TR, W)[:, :H, :]
    )
    return np.ascontiguousarray(out, dtype=np.float32), res


def kernel(grid_spikes, distance_weights):
    out, _ = _run(grid_spikes, distance_weights, trace=False)
    return out


def kernel_traced(grid_spikes, distance_weights):
    out, res = _run(grid_spikes, distance_weights, trace=True)
    return out, res



# revision 4
# speedup vs baseline: 1.1346x; 1.1346x over previous
"""Trainium2 Bass kernel for the LocalConnectivity diamond-ring stencil.

out[b, x, y] = sum_{1<=|dx|+|dy|<=5} w[|dx|+|dy|-1] * in[b, (x+dx)%512, (y+dy)%512]

Strategy (v2: pair-folded horizontal columns)
---------------------------------------------
Data-parallel over batch: 64 samples -> 8 cores x 8 samples. Per sample the
512x512 grid is processed in 5 uniform row-tiles of 103 output rows.

Key identity: the dy=+k and dy=-k stencil columns share the same vertical
band weights, so with av_k[r, y] = x[r, y-k] + x[r, y+k] (horizontal shifts
are free AP offsets on the vector engine) the two dy=+-k PE passes collapse
into ONE banded matmul  psum += Band_k @ av_k.  Per tile the PE now runs
6 matmuls (vs 9 in v1):
  - j=0: dy=0 band on x itself (taps w1..w5 both sides, no shift)
  - j=k (k=1..4): band w_{|dx|+k}, |dx| <= 5-k, applied to av_k
  - j=5: diagonal w5 applied to av_5
Band is ALIGNED (out row 103t+p-5 at psum partition p), contraction 113.

Engine split:
  - TensorE: 6 PSUM-accumulating matmuls/tile, 216 ns each warm.
  - VectorE: 5 av-builds per sample, fp16 tensor_tensor in SBUF with unit
    stride and 4B-aligned reads -> 2x_1P DVE mode (~1.4 us per full-sample
    build of [113, 5, 512]). Even k needs odd read offsets, so those builds
    read at (4-k, 4+k) [both even] and the matmul moving AP reads av_k at
    +1 element (moving APs have no alignment constraint).
  - ScalarE: psum -> SBUF fp16 eviction copies + circular column halos.
  - Output DMA casts fp16 -> f32 (software DGE dge-cast, same as the
    f32 -> fp16 cast on the input DMAs).

Hardware facts this layout is built on (microbenchmarked in v1):
 - gpsimd software-DGE DMAs spanning all 128 SBUF partitions run at
   300-470 GB/s; partial-partition DMAs fall to ~43 GB/s. All bulk
   transfers are full-128-partition DMAs; input host-padded to
   [8, 540, 512]/core so each tile-plane is one clean casting DMA.
 - Each gpsimd dma_start costs ~0.9 us on the Q7; 8 DMA completion
   semaphores recycle round-robin.
 - PE HAM clock gate: cold = 1.2 GHz, warm = 2.4 GHz after ~3.4 us of
   sustained activity. The framework preamble occupies the first ~6.9 us;
   7 dummy matmuls bridge from preamble end to first-data-ready so the
   real stream starts warm.
"""

import numpy as np

import concourse.bass as bass
import concourse.bacc as bacc
import concourse.mybir as mybir
from concourse import tile
from concourse.bass_utils import run_bass_kernel_spmd

B, H, W = 64, 512, 512
NCORES = 8
BPC = B // NCORES  # samples per core
MAXD = 5
HALO = MAXD
TR = 103  # output rows per tile
NT = 5
CTR = TR + 2 * HALO  # 113 contraction rows
XW = W + 2 * HALO  # 522
HPAD = HALO + H + 23  # 540 padded input rows per sample
HOUT = NT * 128  # 640 padded output rows per sample
NJ = 6  # stationaries: dy0 band + G1..G4 bands + G5 diag
AVW = 514  # av tile width for even k (512 + 2 pad)

IN_GROUPS = [(0, 2), (2, 3), (5, 3)]  # (b0, size); staggered to hide wire time
N_DUMMY = 7

F16 = mybir.dt.float16

# av-build read offsets: k odd reads at (5-k, 5+k) [even, even], moving
# offset 0; k even reads at (4-k, 4+k) [even, even], moving offset 1.
AV_SPEC = {}
for _k in range(1, MAXD + 1):
    if _k % 2 == 1:
        AV_SPEC[_k] = (5 - _k, 5 + _k, 512, 0)  # (inA, inB, width, mov_off)
    else:
        AV_SPEC[_k] = (4 - _k, 4 + _k, 514, 1)


def _build_band_weights(dw: np.ndarray) -> np.ndarray:
    """[128, 6*128] fp16: stationary j at cols [128j, 128j+128).

    j=0: dy=0 band  B[p+dx, p] = w_{|dx|},   1 <= |dx| <= 5
    j=k: pair band  B[p+dx, p] = w_{|dx|+k}, |dx| <= 5-k   (k = 1..4)
    j=5: diagonal   B[p, p]    = w_5
    """
    wb = np.zeros((128, NJ, 128), dtype=np.float32)
    p = np.arange(128)
    for j in range(NJ):
        k = j
        if j == 5:
            wb[p, j, p] = dw[MAXD - 1]
            continue
        for dx in range(-(MAXD - k), MAXD - k + 1):
            d = abs(dx) + k
            if not (1 <= d <= MAXD):
                continue
            c = p + dx
            valid = (c >= 0) & (c < 128)
            wb[c[valid], j, p[valid]] = dw[d - 1]
    return np.ascontiguousarray(wb.reshape(128, NJ * 128).astype(np.float16))


_CACHED_NC = None


def _custom_ap(base_ap, dims, extra_offset_elems=0):
    """Build a strided AP: dims = [(stride_elems, size), ...]."""
    s = base_ap.copy()
    s.ap.clear()
    s.ap.extend(dims)
    s.offset = s.offset + extra_offset_elems
    return s


def _build_program():
    f32 = mybir.dt.float32

    nc = bacc.Bacc(None, target_bir_lowering=False)
    x = nc.dram_tensor("x", [BPC, HPAD, W], f32, kind="ExternalInput")
    wb = nc.dram_tensor("wb", [128, NJ * 128], F16, kind="ExternalInput")
    y = nc.dram_tensor("y", [BPC, HOUT, W], f32, kind="ExternalOutput")

    with tile.TileContext(nc) as tc:
        with (
            tc.tile_pool(name="wpool", bufs=1) as wpool,
            tc.tile_pool(name="xpool_a", bufs=1) as xpool_a,
            tc.tile_pool(name="xpool_b", bufs=2) as xpool_b,
            tc.tile_pool(name="avpool", bufs=2) as avpool,
            tc.tile_pool(name="opool", bufs=3) as opool,
            tc.tile_pool(name="pspool", bufs=8, space=bass.MemorySpace.PSUM) as pspool,
        ):
            wtile = wpool.tile([128, NJ * 128], F16, tag="wt")
            nc.gpsimd.dma_start(wtile[:], wb[:])

            # PE warm-up: dummies bridge from framework-preamble end (~6.9us)
            # to first-input-ready (~10.5us); HAM hits 8/8 after ~3.4us busy.
            dummy = wpool.tile([128, 640], F16, tag="dummy")
            nc.vector.memset(dummy[:], 0.0)
            wpt = pspool.tile([128, W], f32, tag="pt")
            for _ in range(N_DUMMY):
                nc.tensor.matmul(wpt[0:TR, :], dummy[0:CTR, 0:TR],
                                 dummy[0:CTR, 64:576], start=True, stop=True)

            sample_xt = {}
            for gi, (pool, (b0, gsz)) in enumerate(
                zip((xpool_a, xpool_b, xpool_b), IN_GROUPS)
            ):
                # xt[p, b, t, 5+y] = xpad[b0+b, 103t + p, y]
                #                  = x_orig[b0+b, (103t - 5 + p) % 512, y]
                xt = pool.tile([128, gsz, NT, XW], F16,
                               tag="xta" if gi == 0 else "xtbc")
                for t in range(NT):
                    src = _custom_ap(
                        x[b0], [(W, 128), (HPAD * W, gsz), (1, W)],
                        extra_offset_elems=TR * t * W,
                    )
                    nc.gpsimd.dma_start(xt[:, :, t, HALO : HALO + W], src)
                    # circular column halos for this plane
                    nc.scalar.copy(
                        xt[0:CTR, :, t, 0:HALO], xt[0:CTR, :, t, W : W + HALO]
                    )
                    nc.scalar.copy(
                        xt[0:CTR, :, t, HALO + W :],
                        xt[0:CTR, :, t, HALO : 2 * HALO],
                    )
                for bi in range(gsz):
                    sample_xt[b0 + bi] = (xt, bi)

            for b in range(BPC):
                xt, bq = sample_xt[b]
                # ---- av builds on DVE: av_k = x<<k + x>>k, fp16 2x mode ----
                avs = {}
                for k in range(1, MAXD + 1):
                    avs[k] = avpool.tile([128, NT, AVW], F16, tag=f"av{k}",
                                         name=f"av{k}")
                if b == 0:
                    # chase arriving planes: per-tile builds
                    for t in range(NT):
                        for k in range(1, MAXD + 1):
                            a0, a1, wdt, _ = AV_SPEC[k]
                            nc.vector.tensor_add(
                                avs[k][0:CTR, t, 0:wdt],
                                xt[0:CTR, bq, t, a0 : a0 + wdt],
                                xt[0:CTR, bq, t, a1 : a1 + wdt],
                            )
                else:
                    for k in range(1, MAXD + 1):
                        a0, a1, wdt, _ = AV_SPEC[k]
                        nc.vector.tensor_add(
                            avs[k][0:CTR, :, 0:wdt],
                            xt[0:CTR, bq, :, a0 : a0 + wdt],
                            xt[0:CTR, bq, :, a1 : a1 + wdt],
                        )

                pts = [pspool.tile([128, W], f32, tag="pt", name=f"pt{t}")
                       for t in range(NT)]

                def moving(j, t):
                    if j == 0:
                        return xt[0:CTR, bq, t, HALO : HALO + W]
                    _, _, _, mo = AV_SPEC[j]
                    return avs[j][0:CTR, t, mo : mo + W]

                if b == 0 or b == BPC - 1:
                    # t-outer: first sample chases planes; last staggers psum
                    # completion so eviction + output drain inside the stream
                    loop = [(j, t) for t in range(NT) for j in range(NJ)]
                else:
                    loop = [(j, t) for j in range(NJ) for t in range(NT)]
                for j, t in loop:
                    nc.tensor.matmul(
                        pts[t][0:108, :],
                        wtile[0:CTR, j * 128 : j * 128 + 108],
                        moving(j, t),
                        start=(j == 0),
                        stop=(j == NJ - 1),
                    )

                # ---- eviction on ScalarE: psum -> fp16 SBUF copy ----
                otb = opool.tile([128, NT, W], F16, tag="otb")
                for t in range(NT):
                    nc.scalar.copy(otb[0:108, t, :], pts[t][0:108, :])

                # ---- full-128-partition output DMAs (cast fp16 -> f32) ----
                if b >= BPC - 2:
                    for lo, hi in ((0, 2), (2, 4), (4, 5)):
                        dstp = _custom_ap(
                            y[b], [(W, 128), (128 * W, hi - lo), (1, W)],
                            extra_offset_elems=lo * 128 * W,
                        )
                        nc.gpsimd.dma_start(dstp, otb[:, lo:hi, :])
                else:
                    dst = _custom_ap(y[b], [(W, 128), (128 * W, NT), (1, W)])
                    nc.gpsimd.dma_start(dst, otb[:, :, :])
    nc.compile()
    return nc


def _get_program():
    global _CACHED_NC
    if _CACHED_NC is None:
        _CACHED_NC = _build_program()
    return _CACHED_NC


def _run(grid_spikes, distance_weights, trace=False):
    grid_spikes = np.ascontiguousarray(np.asarray(grid_spikes, dtype=np.float32))
    distance_weights = np.asarray(distance_weights, dtype=np.float32)
    assert grid_spikes.shape == (B, H, W), grid_spikes.shape
    wb_np = _build_band_weights(distance_weights)

    # pad each sample: 5 wrap rows in front (507..511), 23 behind (0..22)
    xpad = np.concatenate(
        [grid_spikes[:, H - HALO :, :], grid_spikes, grid_spikes[:, :23, :]],
        axis=1,
    )  # [B, 540, W]
    assert xpad.shape[1] == HPAD

    nc = _get_program()
    in_maps = [
        {
            "x": np.ascontiguousarray(xpad[i * BPC : (i + 1) * BPC]),
            "wb": wb_np,
        }
        for i in range(NCORES)
    ]
    res = run_bass_kernel_spmd(nc, in_maps, list(range(NCORES)), trace=trace)
    ypad = np.concatenate(
        [res.results[i]["y"] for i in range(NCORES)], axis=0
    )  # [B, 640, W]
    # unpack: row 103t + p lives at padded row 128t + p (p < 103); the last
    # tile's rows 512..514 are circular duplicates the slice drops.
    out = (
        ypad.reshape(B, NT, 128, W)[:, :, HALO : HALO + TR, :]
        .reshape(B, NT * TR, W)[:, :H, :]
    )
    return np.ascontiguousarray(out, dtype=np.float32), res


def kernel(grid_spikes, distance_weights):
    out, _ = _run(grid_spikes, distance_weights, trace=False)
    return out


def kernel_traced(grid_spikes, distance_weights):
    out, res = _run(grid_spikes, distance_weights, trace=True)
    return out, res
